# revision 1
# baseline (speedup 1.0000x reference)
"""Trainium2 Bass kernel v2 for MllamaTextSelfAttention (B=1, S=2048, HID=4096,
32 Q heads / 8 KV heads, HD=128, RoPE, causal mask, GQA).

Sharding: tensor-parallel over heads across 8 NeuronCores. Core c computes
Q heads [4c, 4c+4) and KV head c, plus the matching slice of the output
projection; the 8 partial outputs are summed on the host.

v2 over baseline (818 us -> ~510 us steady-state):
  - bf16 data path everywhere (fp32 only in PSUM accumulation / den / recip):
    halves DMA traffic, enables FWL weight loads, doubles DVE throughput.
  - single fused pipeline  A0 A1 B0 A2+C0a B1+C0b A3+C1a B2+C1b B3+C2 C3
    so the PE never drains (keeps the HAM p-state at 2.4 GHz):
      A(n)  = QKV projection stripe n (packed wqkv weights, streamed ht)
      B(qs) = attention for query stripe qs, 4 heads software-pipelined
      C(qs) = output projection rows of stripe qs, chains interleaved into
              the following A and B phases to fill their PE slack
  - PSUM packed into exactly 8 banks: tag pA [128,1024]x2 (Q-proj accum /
    score groups), pB [128,1024]x1 (K+V accum / pso+den+psb), pC
    [128,1024]x1 (out-proj accum; the C3 tail rotates pC/pA/pB).
  - exp batched over two PSUM banks per Activation call; diagonal causal
    mask added in one [128,1024] DVE op per group.
  - softmax denominator: e-tiles summed on the (otherwise idle) Pool engine
    into esum (fp32r), then one 512-row PE matmul per (stripe, head) --
    replaces 160 full-size den matmuls on the PE.
  - the reciprocal's psb broadcast + normalization are deferred one
    pipeline step ("fin") so DVE latency never stalls the PE.
  - RoPE(n) is emitted at the end of B(n-1) so its DVE work never queues
    ahead of attention mask-adds.
  - V transposed via DMA xbar transpose (SP queue) instead of PE.
  - y staged to bf16 SBUF (copies split Act/DVE by phase) and written on
    both hardware DGE queues; host sums the 8 partial outputs in fp64.
  - loop_n variant wraps the whole kernel in tc.For_i for dispatch-free
    steady-state timing (SWDGE is rerouted to Act inside the loop).
"""

import math
import os
import sys

for _p in (
    "/opt/trn_rl_repo",
    "/root/.axon_site",
    "/root/.axon_site/_ro/trn_rl_repo",
    "/root/.axon_site/_ro/pypackages",
):
    if os.path.isdir(_p) and _p not in sys.path:
        sys.path.append(_p)

import numpy as np
from contextlib import ExitStack

import concourse.bass as bass
import concourse.tile as tile
from concourse import mybir
from concourse.bass_utils import run_bass_kernel_spmd

try:
    import ml_dtypes

    BF = ml_dtypes.bfloat16
except ImportError:  # pragma: no cover
    import jax.numpy as jnp

    BF = jnp.bfloat16

F32 = mybir.dt.float32
FR = mybir.dt.float32r
BF16 = mybir.dt.bfloat16
ACTF = mybir.ActivationFunctionType

B, S, HID = 1, 2048, 4096
NH, NKV, HD = 32, 8, 128
NCORES = 8
QH = NH // NCORES          # 4 q heads per core
SS = 512                   # sequence stripe
NQS = S // SS              # 4 stripes
NKT = S // 128             # 16 k tiles over full seq
KH = HID // 128            # 32 hidden-dim contraction tiles
NEG = -1e9


def _split_multi_waits(nc: bass.Bass):
    """Walrus in this container encodes at most ONE sync-wait command per
    instruction. Hoist extra waits onto injected same-engine NoOps placed
    immediately before the instruction; engines are in-order so the
    semantics are unchanged."""
    n = 0
    for fn in nc.m.functions:
        for bb in fn.blocks:
            out = []
            for inst in bb.instructions:
                si = inst.sync_info
                if si is not None and si.on_wait and len(si.on_wait) > 1:
                    waits = list(si.on_wait)
                    for w in waits[:-1]:
                        n += 1
                        nop = mybir.InstNoOp(name=f"I-swait-{n}", ins=[], outs=[])
                        nop.engine = inst.engine
                        nop.sync_info = mybir.SyncInfo(on_wait=[w], on_update=[])
                        out.append(nop)
                    si.on_wait = [waits[-1]]
                out.append(inst)
            bb.instructions[:] = out
    return nc


_BUILD_CACHE = {}


def _build(causal: bool, split_waits: bool = True, loop_n=None, ablate_b=False) -> bass.Bass:
    key = (causal, split_waits, loop_n, ablate_b)
    if key in _BUILD_CACHE:
        return _BUILD_CACHE[key]

    nc = bass.Bass()
    hT = nc.dram_tensor("hT", [HID, S], BF16, kind="ExternalInput")
    wqkvT = nc.dram_tensor("wqkvT", [HID, QH * HD + 2 * HD], BF16, kind="ExternalInput")
    woT = nc.dram_tensor("woT", [QH * HD, HID], BF16, kind="ExternalInput")
    cosT = nc.dram_tensor("cosT", [HD, S], BF16, kind="ExternalInput")
    sinT = nc.dram_tensor("sinT", [HD, S], BF16, kind="ExternalInput")
    if causal:
        maskd = nc.dram_tensor("maskd", [128, 4 * SS], BF16, kind="ExternalInput")
    else:
        maskT = nc.dram_tensor("maskT", [S, S], BF16, kind="ExternalInput")
    y = nc.dram_tensor("y", [S, HID], BF16, kind="ExternalOutput")

    with tile.TileContext(nc) as tc, ExitStack() as ctx:
        if loop_n is not None:
            # device-side repeat loop for dispatch-amortized timing; SWDGE
            # (gpsimd) DMA inside For_i fails this walrus' codegen, so the
            # loop variant issues the background prefetches from Act instead
            ctx.enter_context(tc.For_i(0, loop_n, 1))
        bg = nc.scalar if loop_n is not None else nc.gpsimd
        wp = ctx.enter_context(tc.tile_pool(name="wp", bufs=1))
        hp = ctx.enter_context(tc.tile_pool(name="hp", bufs=16))
        vp = ctx.enter_context(tc.tile_pool(name="vp", bufs=2))
        ep = ctx.enter_context(tc.tile_pool(name="ep", bufs=2))
        rp = ctx.enter_context(tc.tile_pool(name="rp", bufs=2))
        pp = ctx.enter_context(tc.tile_pool(name="pp", bufs=1, space="PSUM"))

        # ---- persistent SBUF ----
        # packed per-k projection weights: [q0|q1|q2|q3|k|v] along free dim
        wqkv_c = [wp.tile([128, QH * 128 + 256], BF16, name=f"wqkvc{k}") for k in range(KH)]
        wo_sb = wp.tile([128, QH * HID], BF16)
        cos_sb = wp.tile([128, S], BF16)
        sin_sb = wp.tile([128, S], BF16)
        qT = wp.tile([128, QH * S], BF16)    # [d, (stripe, head, s)]
        kT = wp.tile([128, S], BF16)         # [d, s]
        v_sb = wp.tile([128, S], BF16)       # [s-in-tile, (t, d)]
        ot = wp.tile([128, QH * S], BF16)    # [d, (stripe, head, s)] normalized O^T
        ones_f = wp.tile([128, 128], F32)
        nc.vector.memset(ones_f[:], 1.0)
        ones = wp.tile([128, 128], FR, name="ones_fr")
        nc.vector.tensor_copy(ones[:], ones_f[:])
        if causal:
            md_sb = wp.tile([128, 4 * SS], BF16)
        else:
            mrow = wp.tile([128, NKT * SS], BF16)   # mask row-block for one stripe

        # ---- initial DMAs ----
        # SP: per-k packed weights interleaved with the first hidden stripe so
        # the first matmul can start after ~2 tiles of DMA.
        ht0 = []
        for k in range(KH):
            nc.sync.dma_start(wqkv_c[k][:], wqkvT[k * 128 : (k + 1) * 128, :])
            ht = hp.tile([128, SS], BF16, tag="ht")
            heng = nc.scalar if k % 2 else nc.sync
            heng.dma_start(ht[:], hT[k * 128 : (k + 1) * 128, 0:SS])
            ht0.append(ht)
        # background prefetch on the software-DGE queue
        bg.dma_start(cos_sb[:], cosT[:, :])
        bg.dma_start(sin_sb[:], sinT[:, :])
        if causal:
            bg.dma_start(md_sb[:], maskd[:, :])
        for hh in range(QH):
            bg.dma_start(
                wo_sb[:, hh * HID : (hh + 1) * HID],
                woT[hh * 128 : (hh + 1) * 128, :],
            )

        # ---------------- phase emitters ----------------

        def emit_A(n, interleave=None):
            """QKV projection stripe n + RoPE + V transpose. interleave is an
            iterator of closures (C-phase chains) to emit between k-tiles."""
            psq01 = pp.tile([128, 1024], F32, tag="pA", bufs=2)
            psq23 = pp.tile([128, 1024], F32, tag="pA", bufs=2)
            pskv = pp.tile([128, 1024], F32, tag="pB")
            for k in range(KH):
                if n == 0:
                    ht = ht0[k]
                else:
                    ht = hp.tile([128, SS], BF16, tag="ht")
                    heng = nc.scalar if k % 2 else nc.sync
                    heng.dma_start(
                        ht[:], hT[k * 128 : (k + 1) * 128, n * SS : (n + 1) * SS]
                    )
                w = wqkv_c[k]
                st, sp = (k == 0), (k == KH - 1)
                nc.tensor.matmul(psq01[:, 0:512], w[:, 0:128], ht[:], start=st, stop=sp)
                nc.tensor.matmul(psq01[:, 512:1024], w[:, 128:256], ht[:], start=st, stop=sp)
                nc.tensor.matmul(psq23[:, 0:512], w[:, 256:384], ht[:], start=st, stop=sp)
                nc.tensor.matmul(psq23[:, 512:1024], w[:, 384:512], ht[:], start=st, stop=sp)
                nc.tensor.matmul(pskv[:, 0:512], w[:, 512:640], ht[:], start=st, stop=sp)
                nc.tensor.matmul(pskv[:, 512:1024], w[:, 640:768], ht[:], start=st, stop=sp)
                if interleave is not None and k % 4 == 3:
                    chain = next(interleave, None)
                    if chain is not None:
                        chain()
            base = n * (QH * SS)
            nc.vector.tensor_copy(qT[:, base : base + 1024], psq01[:])
            nc.vector.tensor_copy(qT[:, base + 1024 : base + 2048], psq23[:])
            nc.scalar.copy(kT[:, n * SS : (n + 1) * SS], pskv[:, 0:512])
            vstage = vp.tile([128, SS], BF16, tag="vst")
            nc.scalar.copy(vstage[:], pskv[:, 512:1024])

            # V transpose via DMA xbar (SP hardware queue), [128,128] blocks
            for j in range(4):
                t = 4 * n + j
                nc.sync.dma_start_transpose(
                    v_sb[:, t * 128 : (t + 1) * 128],
                    vstage[:, j * 128 : (j + 1) * 128],
                )

        def emit_rope(n):
            """RoPE on the 4 q-head stripes + the k stripe of stripe n (DVE,
            in place, bf16). Emitted late (after B(n-1)) so the DVE work never
            queues ahead of attention mask-adds."""
            base = n * (QH * SS)
            cs = cos_sb[:, n * SS : (n + 1) * SS]
            sn = sin_sb[:, n * SS : (n + 1) * SS]
            targets = [qT[:, base + m * SS : base + (m + 1) * SS] for m in range(QH)]
            targets.append(kT[:, n * SS : (n + 1) * SS])
            for src in targets:
                rot = rp.tile([128, SS], BF16, tag="rot")
                tmp = rp.tile([128, SS], BF16, tag="tmp")
                nc.vector.tensor_scalar_mul(rot[0:64, :], src[64:128, :], -1.0)
                nc.vector.tensor_copy(rot[64:128, :], src[0:64, :])
                nc.vector.tensor_mul(tmp[:], src, cs)
                nc.vector.tensor_mul(rot[:], rot[:], sn)
                nc.vector.tensor_add(src, tmp[:], rot[:])

        def _pair_order(qs):
            nkt = 4 * qs + 4 if causal else NKT
            allp = list(range(0, nkt, 2))
            if causal:
                diag = [t for t in allp if t >= 4 * qs]
                rest = [t for t in allp if t < 4 * qs]
                return diag + rest
            return allp

        def emit_scores(qs, h):
            """Scores + exp + esum for head h of stripe qs. Returns (e, esum)."""
            nkt = 4 * qs + 4 if causal else NKT
            e = ep.tile([128, NKT * SS], BF16, tag="e")
            esum = ep.tile([128, SS], FR, tag="esum")
            qsl = qT[:, qs * (QH * SS) + h * SS : qs * (QH * SS) + (h + 1) * SS]
            # diagonal (masked) pairs FIRST: their extra DVE mask hop then
            # hides behind later plain pairs, and the final exp gating the
            # pso chain is hop-free
            pairs = _pair_order(qs)
            for gi, t0 in enumerate(pairs):
                pss = pp.tile([128, 1024], F32, tag="pA", bufs=2)
                for half in range(2):
                    t = t0 + half
                    nc.tensor.matmul(
                        pss[:, half * 512 : half * 512 + 512],
                        kT[:, t * 128 : (t + 1) * 128],
                        qsl,
                        start=True,
                        stop=True,
                    )
                if causal and t0 >= 4 * qs:
                    j = t0 - 4 * qs
                    nc.vector.tensor_add(
                        pss[:], pss[:], md_sb[:, j * SS : (j + 2) * SS]
                    )
                elif not causal:
                    nc.vector.tensor_add(
                        pss[:], pss[:], mrow[:, t0 * SS : (t0 + 2) * SS]
                    )
                eg = e[:, t0 * SS : (t0 + 2) * SS]
                nc.scalar.activation(eg, pss[:], ACTF.Exp)
                if gi == 0:
                    nc.gpsimd.tensor_copy(esum[:], e[:, t0 * SS : (t0 + 1) * SS])
                else:
                    nc.gpsimd.tensor_add(
                        esum[:], esum[:], e[:, t0 * SS : (t0 + 1) * SS]
                    )
                nc.gpsimd.tensor_add(
                    esum[:], esum[:], e[:, (t0 + 1) * SS : (t0 + 2) * SS]
                )
            return e, esum

        def emit_psoden(qs, h, e, esum, interleave=None):
            """pso accumulation + den matmul + recip. Returns a finisher
            closure (psb broadcast + normalization) to emit a bit later so
            the DVE recip latency never stalls the PE."""
            nkt = 4 * qs + 4 if causal else NKT
            psB = pp.tile([128, 1024], F32, tag="pB")
            # consume e in the same order scores produced it, and drip C-chain
            # filler between sub-chains so the PE never waits at the exp rate
            ts = [t0 + half for t0 in _pair_order(qs) for half in range(2)]
            for i, t in enumerate(ts):
                nc.tensor.matmul(
                    psB[:, 0:512],
                    v_sb[:, t * 128 : (t + 1) * 128],
                    e[:, t * SS : (t + 1) * SS],
                    start=(i == 0),
                    stop=(i == nkt - 1),
                )
                if interleave is not None and i % 4 == 3 and i != nkt - 1:
                    chain = next(interleave, None)
                    if chain is not None:
                        chain()
            nc.tensor.matmul(
                psB[0:1, 512:1024], ones[:, 0:1], esum[:], start=True, stop=True
            )
            rec = rp.tile([1, SS], FR, tag="rec")
            with nc.allow_low_precision(reason="fp32r recip feeds matmul"):
                nc.vector.reciprocal(rec[:], psB[0:1, 512:1024])

            def fin():
                nc.tensor.matmul(
                    psB[:, 512:1024], ones[0:1, 0:128], rec[:],
                    start=True, stop=True,
                )
                od = ot[:, qs * (QH * SS) + h * SS : qs * (QH * SS) + (h + 1) * SS]
                nc.scalar.copy(od, psB[:, 0:512])
                nc.vector.tensor_mul(od, od, psB[:, 512:1024])

            return fin

        def emit_B(qs, interleave=None, nchunk=4):
            if not causal:
                for t in range(NKT):
                    nc.sync.dma_start(
                        mrow[:, t * SS : (t + 1) * SS],
                        maskT[t * 128 : (t + 1) * 128, qs * SS : (qs + 1) * SS],
                    )

            def chunk():
                if interleave is not None:
                    for _ in range(nchunk):
                        chain = next(interleave, None)
                        if chain is not None:
                            chain()

            prev = None
            for h in range(QH):
                cur = emit_scores(qs, h)
                if prev is not None:
                    fin = emit_psoden(qs, h - 1, *prev, interleave=interleave)
                    chunk()
                    fin()
                prev = cur
            fin = emit_psoden(qs, QH - 1, *prev, interleave=interleave)
            chunk()
            fin()
            if interleave is not None:
                for chain in interleave:
                    chain()
            if qs + 1 < NQS:
                emit_rope(qs + 1)

        def C_chains(qs, tags=("pC",), copy_eng=("vector",)):
            """Output projection for stripe qs as a list of closures.
            copy_eng: engines cycled for the PSUM->SBUF staging copy (the
            Pool engine cannot read PSUM on real hardware)."""
            chains = []
            for st in range(4):
                for nnp in range(4):
                    def chain(st=st, nnp=nnp, i=st * 4 + nnp):
                        tag = tags[i % len(tags)]
                        psy = pp.tile(
                            [128, 1024], F32, tag=tag, bufs=2 if tag == "pA" else None
                        )
                        for half in range(2):
                            nn = nnp * 2 + half
                            for hh in range(QH):
                                nc.tensor.matmul(
                                    psy[:, half * 512 : half * 512 + 512],
                                    ot[:, qs * (QH * SS) + hh * SS + st * 128 : qs * (QH * SS) + hh * SS + (st + 1) * 128],
                                    wo_sb[:, hh * HID + nn * 512 : hh * HID + (nn + 1) * 512],
                                    start=(hh == 0),
                                    stop=(hh == QH - 1),
                                )
                        yt = vp.tile([128, 1024], BF16, tag="yt", bufs=6)
                        eng = copy_eng[i % len(copy_eng)]
                        if eng == "scalar":
                            nc.scalar.copy(yt[:], psy[:])
                        else:
                            nc.vector.tensor_copy(yt[:], psy[:])
                        row = (qs * 4 + st) * 128
                        deng = nc.sync if i % 2 else nc.scalar
                        deng.dma_start(
                            y[row : row + 128, nnp * 1024 : (nnp + 1) * 1024], yt[:]
                        )
                    chains.append(chain)
            return chains

        # ---------------- pipeline ----------------
        if ablate_b:
            # timing diagnostic: A + C phases only (attention removed); ot is
            # filled with a constant so C consumes defined data
            nc.vector.memset(ot[:], 0.01)
            emit_A(0)
            emit_A(1)
            c0 = C_chains(0, copy_eng=("scalar",) * 8 + ("vector",) * 8)
            emit_A(2, interleave=iter(c0[:8]))
            for chain in c0[8:]:
                chain()
            c1 = C_chains(1, copy_eng=("scalar",) * 8 + ("vector",) * 8)
            emit_A(3, interleave=iter(c1[:8]))
            for chain in c1[8:]:
                chain()
            for chain in C_chains(2, copy_eng=("vector",)):
                chain()
            for chain in C_chains(3, tags=("pC", "pA", "pB"), copy_eng=("scalar", "vector")):
                chain()
        else:
            # All C chains go into the B windows: the dense A phases have no
            # PE slack to fill (interleaving there only serializes), while on
            # real hardware B's cross-engine mask->exp->pso chains stall the
            # PE ~3x longer than the cost model predicts.
            emit_A(0)
            emit_rope(0)
            emit_A(1)
            emit_B(0)
            emit_A(2)
            emit_B(1, interleave=iter(C_chains(0, copy_eng=("vector", "scalar"))))
            emit_A(3)
            emit_B(2, interleave=iter(C_chains(1, copy_eng=("vector", "scalar"))))
            emit_B(3, interleave=iter(C_chains(2, copy_eng=("vector", "scalar"))))
            for chain in C_chains(3, tags=("pC", "pA", "pB"), copy_eng=("scalar", "vector")):
                chain()

    if split_waits:
        _split_multi_waits(nc)
    _BUILD_CACHE[key] = nc
    return nc


def _causal_mask_ref() -> np.ndarray:
    return np.triu(np.full((S, S), NEG, np.float32), k=1)


def _diag_mask_tiles() -> np.ndarray:
    p = np.arange(128, dtype=np.int64)[:, None]
    f = np.arange(SS, dtype=np.int64)[None, :]
    cols = [
        np.where(128 * j + p > f, np.float32(NEG), np.float32(0.0)) for j in range(4)
    ]
    return np.ascontiguousarray(np.concatenate(cols, axis=1).astype(np.float32))


def make_in_maps(hidden_states, attention_mask, cos, sin, wq, wk, wv, wo):
    """Host-side sharding/preprocessing. Returns (causal, in_maps)."""
    h = np.ascontiguousarray(np.asarray(hidden_states, dtype=np.float32)[0])
    m2 = np.ascontiguousarray(np.asarray(attention_mask, dtype=np.float32)[0, 0])
    wq = np.asarray(wq, dtype=np.float32)
    wk = np.asarray(wk, dtype=np.float32)
    wv = np.asarray(wv, dtype=np.float32)
    wo = np.asarray(wo, dtype=np.float32)

    causal = bool(np.array_equal(m2, _causal_mask_ref()))
    hT = np.ascontiguousarray(h.T).astype(BF)
    cosT = np.ascontiguousarray(np.asarray(cos, dtype=np.float32)[0].T).astype(BF)
    sinT = np.ascontiguousarray(np.asarray(sin, dtype=np.float32)[0].T).astype(BF)
    sc = np.float32(1.0 / math.sqrt(HD))
    if causal:
        md = _diag_mask_tiles().astype(BF)
    else:
        mT = np.ascontiguousarray(m2.T).astype(BF)

    in_maps = []
    for c in range(NCORES):
        wqkv = np.concatenate(
            [
                (wq[c * QH * HD : (c + 1) * QH * HD] * sc).T,
                wk[c * HD : (c + 1) * HD].T,
                wv[c * HD : (c + 1) * HD].T,
            ],
            axis=1,
        )
        im = {
            "hT": hT,
            "cosT": cosT,
            "sinT": sinT,
            "wqkvT": np.ascontiguousarray(wqkv).astype(BF),
            "woT": np.ascontiguousarray(wo[:, c * QH * HD : (c + 1) * QH * HD].T).astype(BF),
        }
        if causal:
            im["maskd"] = md
        else:
            im["maskT"] = mT
        in_maps.append(im)
    return causal, in_maps


def kernel(hidden_states, attention_mask, cos, sin, wq, wk, wv, wo):
    causal, in_maps = make_in_maps(
        hidden_states, attention_mask, cos, sin, wq, wk, wv, wo
    )
    nc = _build(causal)
    res = run_bass_kernel_spmd(nc, in_maps, list(range(NCORES)))
    out = np.zeros((S, HID), np.float64)
    for c in range(NCORES):
        out += res.results[c]["y"].astype(np.float64)
    return out.reshape(B, S, HID).astype(np.float32)

